# revision 34
# baseline (speedup 1.0000x reference)
"""Trainium2 Bass kernel for nn_MultiHeadedSelfAttention_5179730559275.

Reference math (per batch b):
  q = wq @ x + bq ; k = wk @ x + bk ; v = wv @ x + bv        (1x1 conv, C=256 -> O=256)
  per o-channel (o = head*32 + d), with Q_o,K_o,V_o = 64x64 images [H,W]:
    S_o = Q_o @ K_o^T / sqrt(32); P_o = softmax(S_o, axis=-1); ctx_o = P_o @ V_o

Sharding: data-parallel over batch, 2 batches per core on 8 cores.

Per-core pipeline (per batch):
  1. fp16 x tiles [c, pix] -> PE projections (lhsT = w^T fp16 stationary,
     rhs = x fp16 moving, N=512) -> psum [o', 512]
  2. psum->SBUF copies add bias, cast fp16, and write interleaved layouts
     pairing o with o+128 (om = o chunk):
       q16/k16: [j, h*128 + om*64 + w]   (j = o mod 128)
       v16:     [j, w*128 + om*64 + g]   (bv folded into the v bias: softmax
                                          rows sum to 1, so P@(V+bv) = P@V+bv)
  3. ONE batched xbar DMA transpose per tensor ([128, 64*128] -> [128, 64, 128])
     gives matmul-ready layouts:
       qS/kS: [om*64 + w, h, j]  (per-o transposed images, o-pair stacked)
       vS:    [om*64 + g, w, j]  (natural images + ones column for Z)
  4. Attention per pair j: quadrant matmuls (K=64 at partition bases 0/64):
       S^T psum [om*64+g, h] ; exp (ACT, bias -2) -> eS fp16
       ctx psum [om*64+h, 0:64]=E^T.T@V, col 64 = Z (ones column)
     normalize on DVE: recip Z for a 4-j group, then one broadcasted
     tensor_tensor multiply ctx*(1/Z) -> fp16 out tile, DMA out.
"""

import numpy as np

import concourse.bass as bass
import concourse.bacc as bacc
import concourse.tile as tile
from concourse import mybir
from concourse import bass2jax

NCORES = 8
B, C, H, W = 16, 256, 64, 64
O = 256
PIX = H * W
BL = B // NCORES  # batches per core
SCALE = 1.0 / float(np.sqrt(32.0))
EXP_BIAS = -2.0  # softmax-invariant shift keeping exp() well inside fp16 range

FP32 = mybir.dt.float32
FP16 = mybir.dt.float16


def build_kernel(nc: bass.Bass):
    x_in = nc.declare_dram_parameter("x", [BL, C, PIX], FP16, isOutput=False)
    wT_in = nc.declare_dram_parameter("wT", [3, C, O], FP16, isOutput=False)
    bias_in = nc.declare_dram_parameter("bias", [3, O], FP32, isOutput=False)
    out = nc.declare_dram_parameter("out", [BL, O, PIX], FP16, isOutput=True)

    with tile.TileContext(nc) as tc:
        with (
            tc.tile_pool(name="singles", bufs=1) as singles,
            tc.tile_pool(name="xin", bufs=4) as xpool,
            tc.tile_pool(name="p16", bufs=1) as p16pool,
            tc.tile_pool(name="tsp", bufs=2) as tpool,
            tc.tile_pool(name="small", bufs=8) as small,
            tc.tile_pool(name="psA", bufs=2, space="PSUM") as psA,
            tc.tile_pool(name="psS", bufs=3, space="PSUM") as psS,
            tc.tile_pool(name="psC", bufs=3, space="PSUM") as psC,
        ):
            # ---- constants loaded once ----
            w_sb = singles.tile([128, 3, 2, O], FP16)  # [c', proj, cc, o]
            nc.sync.dma_start(
                out=w_sb,
                in_=wT_in.rearrange("t (cc c) o -> c t cc o", cc=2),
            )
            bias_sb = singles.tile([128, 3, 2], FP32)  # [o', proj, oc]
            nc.sync.dma_start(
                out=bias_sb,
                in_=bias_in.rearrange("t (oc o) -> o t oc", oc=2),
            )
            expb_sb = singles.tile([128, 1], FP32)
            nc.vector.memset(expb_sb, EXP_BIAS)

            tensors = {}

            # prefetch x for ALL batches before any compute: the SDMA engines
            # are idle early, and batch-1 projections must not wait on HBM
            # reads that would otherwise contend with the batch-0 transposes
            xtiles = {}
            for b in range(BL):
                for cc in range(2):
                    xtiles[(b, cc)] = xpool.tile(
                        [128, PIX], FP16, tag="xsb", name=f"xsb_{b}_{cc}"
                    )
            # quarter-image DMAs let the first projection matmuls start while
            # the rest of x is still in flight
            for quarter in range(4):
                px = slice(quarter * (PIX // 4), (quarter + 1) * (PIX // 4))
                for b in range(BL):
                    for cc in range(2):
                        nc.gpsimd.dma_start(
                            out=xtiles[(b, cc)][:, px],
                            in_=x_in[b, cc * 128 : (cc + 1) * 128, px],
                        )

            def front_chunks(b):
                """Emission units for one batch's front: 6 chunks of
                (proj, oc), each 8 matmul-pairs + 8 psum->SBUF copies, with
                the batched transpose attached to the last chunk of each
                projection. Returned as closures so the caller can interleave
                them with attention groups of the previous batch."""
                xsb = [xtiles[(b, 0)], xtiles[(b, 1)]]

                q16 = p16pool.tile([128, H, 2, W], FP16, tag="q16")  # [j, h, om, w]
                k16 = p16pool.tile([128, H, 2, W], FP16, tag="k16")
                v16 = p16pool.tile([128, W, 2, H], FP16, tag="v16")  # [j, w, om, g]

                qS = tpool.tile([128, H, 128], FP16, tag="qS")  # [om*64 + w, h, j]
                kS = tpool.tile([128, H, 128], FP16, tag="kS")
                vS = tpool.tile([128, W + 1, 128], FP16, tag="vS")
                tensors[b] = (qS, kS, vS)

                def chunk(proj, oc):
                    if proj == 2 and oc == 0:
                        nc.vector.memset(vS[:, W, :], 1.0)
                    for nt in range(8):
                        ps = psA.tile([128, 512], FP32, tag="ps_proj")
                        for cc in range(2):
                            nc.tensor.matmul(
                                ps,
                                lhsT=w_sb[:, proj, cc, oc * 128 : (oc + 1) * 128],
                                rhs=xsb[cc][:, nt * 512 : (nt + 1) * 512],
                                start=(cc == 0),
                                stop=(cc == 1),
                            )
                        bias_ap = bias_sb[:, proj, oc : oc + 1]
                        if proj == 0:  # q
                            nc.scalar.activation(
                                out=q16[:, nt * 8 : (nt + 1) * 8, oc, :],
                                in_=ps.rearrange("p (h w) -> p h w", w=W),
                                func=mybir.ActivationFunctionType.Identity,
                                bias=bias_ap,
                                scale=1.0,
                            )
                        elif proj == 1:  # k
                            nc.vector.tensor_scalar_add(
                                out=k16[:, nt * 8 : (nt + 1) * 8, oc, :],
                                in0=ps.rearrange("p (h w) -> p h w", w=W),
                                scalar1=bias_ap,
                            )
                        else:  # v
                            # split across ACT/DVE for engine balance; out
                            # keeps 8-elem contiguous runs (g inner), the
                            # awkward stride goes on the psum read side
                            if oc == 0:
                                nc.scalar.activation(
                                    out=v16[:, :, oc, nt * 8 : (nt + 1) * 8],
                                    in_=ps.rearrange("p (g w) -> p w g", w=W),
                                    func=mybir.ActivationFunctionType.Identity,
                                    bias=bias_ap,
                                    scale=1.0,
                                )
                            else:
                                nc.vector.tensor_scalar_add(
                                    out=v16[:, :, oc, nt * 8 : (nt + 1) * 8],
                                    in0=ps.rearrange("p (g w) -> p w g", w=W),
                                    scalar1=bias_ap,
                                )
                    if oc == 1:
                        # batched xbar transpose: one instruction does all 64
                        # 128x128 tiles of the finished tensor (mid dim =
                        # batch). All on the sync ring: two concurrent xbar
                        # transposes on separate HWDGE rings corrupt data.
                        if proj == 0:
                            nc.sync.dma_start_transpose(
                                out=qS, in_=q16.rearrange("p h om w -> p (h om w)")
                            )
                        elif proj == 1:
                            nc.sync.dma_start_transpose(
                                out=kS, in_=k16.rearrange("p h om w -> p (h om w)")
                            )
                        else:
                            nc.sync.dma_start_transpose(
                                out=vS[:, 0:W, :],
                                in_=v16.rearrange("p w om g -> p (w om g)"),
                            )

                return [
                    (lambda proj=proj, oc=oc: chunk(proj, oc))
                    for proj in range(3)
                    for oc in range(2)
                ]

            JG = 8
            PG = 4

            def emit_S(b, jg):
                qS, kS, vS = tensors[b]
                sp8f = psS.tile([128, 512], FP32, tag="sp8")
                sp8 = sp8f.rearrange("p (i h) -> p i h", h=H)
                for i in range(JG):
                    j = jg + i
                    for om in range(2):
                        pr = slice(om * 64, om * 64 + 64)
                        nc.tensor.matmul(
                            sp8[pr, i, :],
                            lhsT=kS[pr, :, j],
                            rhs=qS[pr, :, j],
                            start=True,
                            stop=True,
                        )
                eS8 = small.tile([128, JG, H], FP16, tag="eS8")
                nc.scalar.activation(
                    out=eS8,
                    in_=sp8,
                    func=mybir.ActivationFunctionType.Exp,
                    bias=expb_sb,
                    scale=1.0,
                )
                return eS8

            def emit_ctx(b, jg, eS8):
                _, _, vS = tensors[b]
                oc8 = small.tile([128, JG, W], FP16, tag="oc8")
                for sg in range(jg, jg + JG, PG):
                    cp4f = psC.tile([128, 512], FP32, tag="cp4")
                    cp4 = cp4f[:, 0 : PG * (W + 1)].rearrange(
                        "p (i c) -> p i c", c=W + 1
                    )
                    for i in range(PG):
                        j = sg + i
                        for om in range(2):
                            pr = slice(om * 64, om * 64 + 64)
                            nc.tensor.matmul(
                                cp4[pr, i, :],
                                lhsT=eS8[pr, j - jg, :],
                                rhs=vS[pr, :, j],
                                start=True,
                                stop=True,
                            )
                    rz4 = small.tile([128, PG], FP32, tag="rz4")
                    nc.vector.reciprocal(out=rz4, in_=cp4[:, :, W])
                    nc.vector.tensor_tensor(
                        oc8[:, sg - jg : sg - jg + PG, :],
                        cp4[:, :, 0:W],
                        rz4[:, :, None].to_broadcast((128, PG, W)),
                        mybir.AluOpType.mult,
                    )
                # out-writes split across the gpsimd (SWDGE) and sync (HWDGE)
                # queues so neither serializes the tail
                for om, eng in ((0, nc.gpsimd), (1, nc.sync)):
                    eng.dma_start(
                        out=out[b, om * 128 + jg : om * 128 + jg + JG, :].rearrange(
                            "j (h w) -> h j w", w=W
                        ),
                        in_=oc8[om * 64 : om * 64 + 64, :, :],
                    )

            # Emission plan: front(0) fully, then front(1) chunks interleaved
            # with attn(0) groups so no engine FIFO head-of-line blocks a
            # whole phase behind another. Attention itself is software-
            # pipelined: S(g+1) queues on TensorE before ctx(g) so the FIFO
            # never stalls waiting on exp(g) (which runs on ACT).
            for ch in front_chunks(0):
                ch()
            fc1 = front_chunks(1)

            prev = None

            def emit_group(b, jg):
                nonlocal prev
                eS8 = emit_S(b, jg)
                if prev is not None:
                    emit_ctx(*prev)
                prev = (b, jg, eS8)

            # tokens: 6 F-chunks woven into the 16 attn(0) groups, front-
            # loaded (F before A) because attn(0) also waits on the b0
            # transposes which land during the first chunks.
            plan = ["F", "F", "A", "F", "A", "F", "A", "A", "F", "A", "A", "F"]
            fi = ai = 0
            for tok in plan:
                if tok == "F":
                    fc1[fi]()
                    fi += 1
                else:
                    emit_group(0, ai * JG)
                    ai += 1
            while ai < 16:
                emit_group(0, ai * JG)
                ai += 1
            for jg in range(0, 128, JG):
                emit_group(1, jg)
            emit_ctx(*prev)
    return nc


_NC_CACHE = {}


def get_nc():
    if "nc" not in _NC_CACHE:
        nc = bacc.Bacc(None, target_bir_lowering=False)
        build_kernel(nc)
        nc.finalize()
        _NC_CACHE["nc"] = nc
    return _NC_CACHE["nc"]


def prep_in_maps(x, wq, bq, wk, bk, wv, bv):
    wT = np.stack(
        [
            np.ascontiguousarray((wq * SCALE).T),
            np.ascontiguousarray(wk.T),
            np.ascontiguousarray(wv.T),
        ]
    ).astype(np.float16)
    biases = np.stack([bq * SCALE, bk, bv]).astype(np.float32)
    xs = np.ascontiguousarray(x.reshape(NCORES, BL, C, PIX)).astype(np.float16)
    return [{"x": xs[i], "wT": wT, "bias": biases} for i in range(NCORES)]


def kernel(x, wq, bq, wk, bk, wv, bv):
    nc = get_nc()
    in_maps = prep_in_maps(x, wq, bq, wk, bk, wv, bv)
    results = bass2jax.run_bass_via_pjrt(nc, in_maps, n_cores=NCORES)
    outs = [np.asarray(r["out"]).reshape(BL, O, H, W) for r in results]
    return np.concatenate(outs, axis=0).astype(np.float32)


# revision 35
# speedup vs baseline: 1.1318x; 1.1318x over previous
"""Trainium2 Bass kernel for nn_MultiHeadedSelfAttention_5179730559275.

Reference math (per batch b):
  q = wq @ x + bq ; k = wk @ x + bk ; v = wv @ x + bv        (1x1 conv, C=256 -> O=256)
  per o-channel (o = head*32 + d), with Q_o,K_o,V_o = 64x64 images [H,W]:
    S_o = Q_o @ K_o^T / sqrt(32); P_o = softmax(S_o, axis=-1); ctx_o = P_o @ V_o

Sharding: data-parallel over batch, 2 batches per core on 8 cores.

Per-core pipeline (per batch):
  1. fp16 x tiles [c, pix] -> PE projections (lhsT = w^T fp16 stationary,
     rhs = x fp16 moving, N=512) -> psum [o', 512]
  2. psum->SBUF copies add bias, cast fp16, and write interleaved layouts
     pairing o with o+128 (om = o chunk):
       q16/k16: [j, h*128 + om*64 + w]   (j = o mod 128)
       v16:     [j, w*128 + om*64 + g]   (bv folded into the v bias: softmax
                                          rows sum to 1, so P@(V+bv) = P@V+bv)
  3. ONE batched xbar DMA transpose per tensor ([128, 64*128] -> [128, 64, 128])
     gives matmul-ready layouts:
       qS/kS: [om*64 + w, h, j]  (per-o transposed images, o-pair stacked)
       vS:    [om*64 + g, w, j]  (natural images + ones column for Z)
  4. Attention per pair j: quadrant matmuls (K=64 at partition bases 0/64):
       S^T psum [om*64+g, h] ; exp (ACT, bias -2) -> eS fp16
       ctx psum [om*64+h, 0:64]=E^T.T@V, col 64 = Z (ones column)
     normalize on DVE: recip Z for a 4-j group, then one broadcasted
     tensor_tensor multiply ctx*(1/Z) -> fp16 out tile, DMA out.
"""

import numpy as np

import concourse.bass as bass
import concourse.bacc as bacc
import concourse.tile as tile
from concourse import mybir
from concourse import bass2jax

NCORES = 8
B, C, H, W = 16, 256, 64, 64
O = 256
PIX = H * W
BL = B // NCORES  # batches per core
SCALE = 1.0 / float(np.sqrt(32.0))
EXP_BIAS = -2.0  # softmax-invariant shift keeping exp() well inside fp16 range

FP32 = mybir.dt.float32
FP16 = mybir.dt.float16


def build_kernel(nc: bass.Bass):
    x_in = nc.declare_dram_parameter("x", [BL, C, PIX], FP16, isOutput=False)
    wT_in = nc.declare_dram_parameter("wT", [3, C, O], FP16, isOutput=False)
    bias_in = nc.declare_dram_parameter("bias", [3, O], FP32, isOutput=False)
    out = nc.declare_dram_parameter("out", [BL, O, PIX], FP16, isOutput=True)

    with tile.TileContext(nc) as tc:
        with (
            tc.tile_pool(name="singles", bufs=1) as singles,
            tc.tile_pool(name="xin", bufs=4) as xpool,
            tc.tile_pool(name="p16", bufs=1) as p16pool,
            tc.tile_pool(name="tsp", bufs=2) as tpool,
            tc.tile_pool(name="small", bufs=8) as small,
            tc.tile_pool(name="psA", bufs=2, space="PSUM") as psA,
            tc.tile_pool(name="psS", bufs=3, space="PSUM") as psS,
            tc.tile_pool(name="psC", bufs=3, space="PSUM") as psC,
        ):
            # ---- constants loaded once ----
            w_sb = singles.tile([128, 3, 2, O], FP16)  # [c', proj, cc, o]
            nc.sync.dma_start(
                out=w_sb,
                in_=wT_in.rearrange("t (cc c) o -> c t cc o", cc=2),
            )
            bias_sb = singles.tile([128, 3, 2], FP32)  # [o', proj, oc]
            nc.sync.dma_start(
                out=bias_sb,
                in_=bias_in.rearrange("t (oc o) -> o t oc", oc=2),
            )
            expb_sb = singles.tile([128, 1], FP32)
            nc.vector.memset(expb_sb, EXP_BIAS)

            tensors = {}

            # prefetch x for ALL batches before any compute: the SDMA engines
            # are idle early, and batch-1 projections must not wait on HBM
            # reads that would otherwise contend with the batch-0 transposes
            xtiles = {}
            for b in range(BL):
                for cc in range(2):
                    xtiles[(b, cc)] = xpool.tile(
                        [128, PIX], FP16, tag="xsb", name=f"xsb_{b}_{cc}"
                    )
            # half-image DMAs let the first projection matmuls start while the
            # rest of x is still in flight
            for half in range(2):
                px = slice(half * (PIX // 2), (half + 1) * (PIX // 2))
                for b in range(BL):
                    for cc in range(2):
                        nc.gpsimd.dma_start(
                            out=xtiles[(b, cc)][:, px],
                            in_=x_in[b, cc * 128 : (cc + 1) * 128, px],
                        )

            def front_chunks(b):
                """Emission units for one batch's front: 6 chunks of
                (proj, oc), each 8 matmul-pairs + 8 psum->SBUF copies, with
                the batched transpose attached to the last chunk of each
                projection. Returned as closures so the caller can interleave
                them with attention groups of the previous batch."""
                xsb = [xtiles[(b, 0)], xtiles[(b, 1)]]

                q16 = p16pool.tile([128, H, 2, W], FP16, tag="q16")  # [j, h, om, w]
                k16 = p16pool.tile([128, H, 2, W], FP16, tag="k16")
                v16 = p16pool.tile([128, W, 2, H], FP16, tag="v16")  # [j, w, om, g]

                qS = tpool.tile([128, H, 128], FP16, tag="qS")  # [om*64 + w, h, j]
                kS = tpool.tile([128, H, 128], FP16, tag="kS")
                vS = tpool.tile([128, W + 1, 128], FP16, tag="vS")
                tensors[b] = (qS, kS, vS)

                def chunk(proj, oc):
                    if proj == 2 and oc == 0:
                        nc.vector.memset(vS[:, W, :], 1.0)
                    for nt in range(8):
                        ps = psA.tile([128, 512], FP32, tag="ps_proj")
                        for cc in range(2):
                            nc.tensor.matmul(
                                ps,
                                lhsT=w_sb[:, proj, cc, oc * 128 : (oc + 1) * 128],
                                rhs=xsb[cc][:, nt * 512 : (nt + 1) * 512],
                                start=(cc == 0),
                                stop=(cc == 1),
                            )
                        bias_ap = bias_sb[:, proj, oc : oc + 1]
                        if proj == 0:  # q
                            nc.scalar.activation(
                                out=q16[:, nt * 8 : (nt + 1) * 8, oc, :],
                                in_=ps.rearrange("p (h w) -> p h w", w=W),
                                func=mybir.ActivationFunctionType.Identity,
                                bias=bias_ap,
                                scale=1.0,
                            )
                        elif proj == 1:  # k
                            nc.vector.tensor_scalar_add(
                                out=k16[:, nt * 8 : (nt + 1) * 8, oc, :],
                                in0=ps.rearrange("p (h w) -> p h w", w=W),
                                scalar1=bias_ap,
                            )
                        else:  # v
                            # split across ACT/DVE for engine balance; out
                            # keeps 8-elem contiguous runs (g inner), the
                            # awkward stride goes on the psum read side
                            if oc == 0:
                                nc.scalar.activation(
                                    out=v16[:, :, oc, nt * 8 : (nt + 1) * 8],
                                    in_=ps.rearrange("p (g w) -> p w g", w=W),
                                    func=mybir.ActivationFunctionType.Identity,
                                    bias=bias_ap,
                                    scale=1.0,
                                )
                            else:
                                nc.vector.tensor_scalar_add(
                                    out=v16[:, :, oc, nt * 8 : (nt + 1) * 8],
                                    in0=ps.rearrange("p (g w) -> p w g", w=W),
                                    scalar1=bias_ap,
                                )
                    if oc == 1:
                        # batched xbar transpose: one instruction does all 64
                        # 128x128 tiles of the finished tensor (mid dim =
                        # batch). All on the sync ring: two concurrent xbar
                        # transposes on separate HWDGE rings corrupt data.
                        if proj == 0:
                            nc.sync.dma_start_transpose(
                                out=qS, in_=q16.rearrange("p h om w -> p (h om w)")
                            )
                        elif proj == 1:
                            nc.sync.dma_start_transpose(
                                out=kS, in_=k16.rearrange("p h om w -> p (h om w)")
                            )
                        else:
                            nc.sync.dma_start_transpose(
                                out=vS[:, 0:W, :],
                                in_=v16.rearrange("p w om g -> p (w om g)"),
                            )

                return [
                    (lambda proj=proj, oc=oc: chunk(proj, oc))
                    for proj in range(3)
                    for oc in range(2)
                ]

            JG = 8
            PG = 4

            def emit_S(b, jg):
                qS, kS, vS = tensors[b]
                sp8f = psS.tile([128, 512], FP32, tag="sp8")
                sp8 = sp8f.rearrange("p (i h) -> p i h", h=H)
                for i in range(JG):
                    j = jg + i
                    for om in range(2):
                        pr = slice(om * 64, om * 64 + 64)
                        nc.tensor.matmul(
                            sp8[pr, i, :],
                            lhsT=kS[pr, :, j],
                            rhs=qS[pr, :, j],
                            start=True,
                            stop=True,
                        )
                eS8 = small.tile([128, JG, H], FP16, tag="eS8")
                nc.scalar.activation(
                    out=eS8,
                    in_=sp8,
                    func=mybir.ActivationFunctionType.Exp,
                    bias=expb_sb,
                    scale=1.0,
                )
                return eS8

            def emit_ctx(b, jg, eS8):
                _, _, vS = tensors[b]
                oc8 = small.tile([128, JG, W], FP16, tag="oc8")
                for sg in range(jg, jg + JG, PG):
                    cp4f = psC.tile([128, 512], FP32, tag="cp4")
                    cp4 = cp4f[:, 0 : PG * (W + 1)].rearrange(
                        "p (i c) -> p i c", c=W + 1
                    )
                    for i in range(PG):
                        j = sg + i
                        for om in range(2):
                            pr = slice(om * 64, om * 64 + 64)
                            nc.tensor.matmul(
                                cp4[pr, i, :],
                                lhsT=eS8[pr, j - jg, :],
                                rhs=vS[pr, :, j],
                                start=True,
                                stop=True,
                            )
                    rz4 = small.tile([128, PG], FP32, tag="rz4")
                    nc.vector.reciprocal(out=rz4, in_=cp4[:, :, W])
                    nc.vector.tensor_tensor(
                        oc8[:, sg - jg : sg - jg + PG, :],
                        cp4[:, :, 0:W],
                        rz4[:, :, None].to_broadcast((128, PG, W)),
                        mybir.AluOpType.mult,
                    )
                # out-writes split across the gpsimd (SWDGE) and sync (HWDGE)
                # queues so neither serializes the tail
                for om, eng in ((0, nc.gpsimd), (1, nc.sync)):
                    eng.dma_start(
                        out=out[b, om * 128 + jg : om * 128 + jg + JG, :].rearrange(
                            "j (h w) -> h j w", w=W
                        ),
                        in_=oc8[om * 64 : om * 64 + 64, :, :],
                    )

            # Emission plan: front(0) fully, then front(1) chunks interleaved
            # with attn(0) groups so no engine FIFO head-of-line blocks a
            # whole phase behind another. Attention itself is software-
            # pipelined: S(g+1) queues on TensorE before ctx(g) so the FIFO
            # never stalls waiting on exp(g) (which runs on ACT).
            for ch in front_chunks(0):
                ch()
            fc1 = front_chunks(1)

            prev = None

            def emit_group(b, jg):
                nonlocal prev
                eS8 = emit_S(b, jg)
                if prev is not None:
                    emit_ctx(*prev)
                prev = (b, jg, eS8)

            # tokens: 6 F-chunks woven into the 16 attn(0) groups, front-
            # loaded (F before A) because attn(0) also waits on the b0
            # transposes which land during the first chunks.
            plan = ["F", "F", "A", "F", "A", "F", "A", "A", "F", "A", "A", "F"]
            fi = ai = 0
            for tok in plan:
                if tok == "F":
                    fc1[fi]()
                    fi += 1
                else:
                    emit_group(0, ai * JG)
                    ai += 1
            while ai < 16:
                emit_group(0, ai * JG)
                ai += 1
            for jg in range(0, 128, JG):
                emit_group(1, jg)
            emit_ctx(*prev)
    return nc


_NC_CACHE = {}


def get_nc():
    if "nc" not in _NC_CACHE:
        nc = bacc.Bacc(None, target_bir_lowering=False)
        build_kernel(nc)
        nc.finalize()
        _NC_CACHE["nc"] = nc
    return _NC_CACHE["nc"]


def prep_in_maps(x, wq, bq, wk, bk, wv, bv):
    wT = np.stack(
        [
            np.ascontiguousarray((wq * SCALE).T),
            np.ascontiguousarray(wk.T),
            np.ascontiguousarray(wv.T),
        ]
    ).astype(np.float16)
    biases = np.stack([bq * SCALE, bk, bv]).astype(np.float32)
    xs = np.ascontiguousarray(x.reshape(NCORES, BL, C, PIX)).astype(np.float16)
    return [{"x": xs[i], "wT": wT, "bias": biases} for i in range(NCORES)]


def kernel(x, wq, bq, wk, bk, wv, bv):
    nc = get_nc()
    in_maps = prep_in_maps(x, wq, bq, wk, bk, wv, bv)
    results = bass2jax.run_bass_via_pjrt(nc, in_maps, n_cores=NCORES)
    outs = [np.asarray(r["out"]).reshape(BL, O, H, W) for r in results]
    return np.concatenate(outs, axis=0).astype(np.float32)
